# revision 21
# baseline (speedup 1.0000x reference)
"""Trainium2 Bass kernel for the Performer-style random-feature map:

    out[n, s] = exp(-||x_n||^2 / 2) * S^{-1/2} * exp((x @ W.T)[n, s] + b[s])
              = exp((x @ W.T)[n, s] + b[s] - 0.5*||x_n||^2 - 0.5*ln(S))

Sharding: data-parallel over the N (row) axis across 8 NeuronCores; W and b
replicated.  Each core computes a [2048, 2048] output block.  Pure SPMD, no
collectives.

Per-core structure (sizes hardcoded for N=16384, D=1024, S=2048):
  - x and W live in SBUF as fp8e4 k-pair strips; matmuls run in DoubleRow
    perf mode (contraction 256/instr, 2x bf16 throughput).  x is laid out
    per 128-row block so each block's operands arrive in one small DMA.
  - each PSUM bank is *pre-loaded with b* by a DVE copy, and the matmuls
    accumulate on top (start=False), so no separate exp(b) multiply pass
    or extra PE work is needed.
  - row norms run on the Activation engine (Square + accum_out) with a tiny
    DVE affine fold to bias_n = -0.5*||x_n||^2 - 0.5*ln(S); ACT exp(psum +
    bias_n) with the per-partition bias port then emits the final bf16 tile.
    The PE's fp8 stream is the only saturated engine.
"""

import sys
from contextlib import ExitStack

if "/opt/trn_rl_repo" not in sys.path:
    sys.path.insert(0, "/opt/trn_rl_repo")

import numpy as np

import concourse.bacc as bacc
import concourse.bass as bass
import concourse.tile as tile
from concourse import mybir

P = 128          # SBUF partitions
N_FULL = 16384   # total rows
D_FULL = 1024    # contraction dim
S_FULL = 2048    # output features
N_CORES = 8
NC_FULL = N_FULL // N_CORES  # rows per core

F32 = mybir.dt.float32
BF16 = mybir.dt.bfloat16
FP8 = mybir.dt.float8e4


def build_nc(NCc=NC_FULL, D=D_FULL, S=S_FULL, warmup=14, xn_early=4):
    """Build the single-core Bass program (same program runs SPMD on 8 cores)."""
    nc = bacc.Bacc("TRN2", target_bir_lowering=False, debug=False)

    KT = D // P            # 8 k strips of 128
    KP = KT // 2           # 4 double-row k pairs
    NB = NCc // P          # 16 row blocks
    NS = 512               # psum bank width (f32)
    CS = S // NS           # 4 psum chunks per row block
    PW = 2 * NS            # psum tile width (2 banks) -> ACT tile size
    neg_half_ln_s = float(-0.5 * np.log(S))

    xb = nc.dram_tensor("xb", [NB, P, KT, P], FP8, kind="ExternalInput").ap()
    xn = nc.dram_tensor("xn", [NCc, D], BF16, kind="ExternalInput").ap()
    w = nc.dram_tensor("w", [P, KT, S], FP8, kind="ExternalInput").ap()
    bv = nc.dram_tensor("bvec", [S], BF16, kind="ExternalInput").ap()
    out = nc.dram_tensor("out", [NCc, S], BF16, kind="ExternalOutput").ap()

    with tile.TileContext(nc) as tc, ExitStack() as ctx:
        singles = ctx.enter_context(tc.tile_pool(name="singles", bufs=1))
        x_nb = [singles.tile([P, KT, P], FP8, tag=f"x{nb}", name=f"x{nb}")
                for nb in range(NB)]
        w_kp = [singles.tile([P, 2, S], FP8, tag=f"w{j}", name=f"w{j}")
                for j in range(KP)]
        b_bc = singles.tile([P, S], BF16)
        bias_tiles = [
            singles.tile([P, 1], F32, tag=f"bias{nb}", name=f"bias{nb}")
            for nb in range(NB)
        ]

        # scalar ring: b broadcast + xn blocks (norm operands)
        xn_pool = ctx.enter_context(tc.tile_pool(name="xnp", bufs=4))
        sq_pool = ctx.enter_context(tc.tile_pool(name="sqp", bufs=2))
        r_pool = ctx.enter_context(tc.tile_pool(name="rp", bufs=2))
        xn_tiles = {}

        def load_xn(nb):
            xt = xn_pool.tile([P, D], BF16, tag="xns", name=f"xn{nb}")
            nc.scalar.dma_start(xt, xn[nb * P:(nb + 1) * P, :])
            xn_tiles[nb] = xt

        bv_bcast = bass.AP(tensor=bv.tensor, offset=bv.offset,
                           ap=[[0, P]] + list(bv.ap))
        # scalar ring: b broadcast (gates the first psum inits), half of W,
        # then the xn stream.
        nc.scalar.dma_start(b_bc, bv_bcast)
        nc.scalar.dma_start(w_kp[1], w[:, 2:4, :])
        nc.scalar.dma_start(w_kp[3], w[:, 6:8, :])
        for nb in range(min(xn_early, NB)):
            load_xn(nb)

        # sync ring: block 0's x, the other half of W, remaining x blocks;
        # out DMAs follow later in program order.  Splitting W across both
        # hardware DGE rings halves its arrival time in the ramp-limited
        # head phase (block 0 consumes every k strip within ~4us).
        nc.sync.dma_start(x_nb[0], xb[0])
        nc.sync.dma_start(w_kp[0], w[:, 0:2, :])
        nc.sync.dma_start(w_kp[2], w[:, 4:6, :])
        for nb in range(1, NB):
            nc.sync.dma_start(x_nb[nb], xb[nb])

        def r_bias(nb):
            xt = xn_tiles[nb]
            sq = sq_pool.tile([P, D], BF16)
            nc.gpsimd.tensor_mul(sq, xt, xt)
            r_raw = r_pool.tile([P, 1], F32)
            nc.vector.tensor_reduce(
                r_raw, sq, axis=mybir.AxisListType.X, op=mybir.AluOpType.add)
            nc.vector.tensor_scalar(
                out=bias_tiles[nb], in0=r_raw,
                scalar1=-0.5, scalar2=neg_half_ln_s,
                op0=mybir.AluOpType.mult, op1=mybir.AluOpType.add)

        psum_pool = ctx.enter_context(
            tc.tile_pool(name="psum", bufs=4, space="PSUM"))
        out_pool = ctx.enter_context(tc.tile_pool(name="osb", bufs=3))

        if warmup:
            # keep the PE busy (p-state ramp) while the first operands land
            dummy_x = singles.tile([P, 2, P], FP8)
            dummy_w = singles.tile([P, 2, NS], FP8)
            nc.vector.memset(dummy_x, 0.0)
            nc.vector.memset(dummy_w, 0.0)
            for i in range(warmup):
                wps = psum_pool.tile([P, PW], F32, tag="ps", name=f"warm{i}")
                nc.tensor.matmul(wps[:, 0:NS], lhsT=dummy_x, rhs=dummy_w,
                                 start=True, stop=True,
                                 perf_mode=mybir.MatmulPerfMode.DoubleRow)

        for nb in range(min(2, NB)):
            r_bias(nb)

        # psum init, one block ahead of use: tile0 <- b via DVE copy,
        # tile1 <- b via ACT copy; the block's matmuls accumulate on top
        # (start=False), so no separate exp(b) multiply pass exists.
        ps_tiles = {}

        def alloc_and_init(nb):
            t0 = psum_pool.tile([P, PW], F32, tag="ps", name=f"ps{nb}_0")
            t1 = psum_pool.tile([P, PW], F32, tag="ps", name=f"ps{nb}_1")
            ps_tiles[nb] = (t0, t1)
            nc.vector.tensor_scalar_mul(t0, b_bc[:, 0:PW], 1.0)
            nc.scalar.activation(
                t1, b_bc[:, PW:2 * PW],
                func=mybir.ActivationFunctionType.Copy)

        alloc_and_init(0)

        for nb in range(NB):
            nxt = nb + xn_early
            if nxt < NB:
                load_xn(nxt)
            if nb + 2 < NB:
                r_bias(nb + 2)
            ps2 = ps_tiles.pop(nb)

            def bank(c):
                return ps2[c // 2][:, (c % 2) * NS:(c % 2 + 1) * NS]

            for j in range(KP):
                lhsT = x_nb[nb][:, 2 * j:2 * j + 2, :]
                for c in range(CS):
                    nc.tensor.matmul(
                        bank(c),
                        lhsT=lhsT,
                        rhs=w_kp[j][:, :, c * NS:(c + 1) * NS],
                        start=False,
                        stop=(j == KP - 1),
                        perf_mode=mybir.MatmulPerfMode.DoubleRow,
                        skip_group_check=True,
                    )
            if nb + 1 < NB:
                alloc_and_init(nb + 1)
            o_sb = out_pool.tile([P, S], BF16)
            # finer ACT/DMA granularity on the last block shortens the tail
            ew = NS if nb == NB - 1 else PW
            for e0 in range(0, S, ew):
                nc.scalar.activation(
                    o_sb[:, e0:e0 + ew],
                    ps2[e0 // PW][:, e0 % PW:e0 % PW + ew],
                    func=mybir.ActivationFunctionType.Exp,
                    bias=bias_tiles[nb],
                    scale=1.0,
                )
                nc.sync.dma_start(
                    out[nb * P:(nb + 1) * P, e0:e0 + ew],
                    o_sb[:, e0:e0 + ew])

    nc.compile()
    return nc


_NC_CACHE = {}


def _get_nc(**kwargs):
    key = tuple(sorted(kwargs.items()))
    if key not in _NC_CACHE:
        _NC_CACHE[key] = build_nc(**kwargs)
    return _NC_CACHE[key]


def make_in_maps(x, W, b):
    import ml_dtypes
    fp8 = ml_dtypes.float8_e4m3
    bf16 = ml_dtypes.bfloat16
    KT = D_FULL // P
    NB = NC_FULL // P
    wT = np.ascontiguousarray(
        W.T.astype(np.float32).reshape(KT, P, S_FULL)
        .transpose(1, 0, 2).astype(fp8))
    bh = np.ascontiguousarray(b.astype(bf16))
    in_maps = []
    for i in range(N_CORES):
        xs = np.asarray(x[i * NC_FULL:(i + 1) * NC_FULL], dtype=np.float32)
        # xb[nb, p, k, m] = xs[nb*128 + m, k*128 + p]
        xbs = np.ascontiguousarray(
            xs.reshape(NB, P, KT, P).transpose(0, 3, 2, 1).astype(fp8))
        in_maps.append({
            "xb": xbs,
            "xn": np.ascontiguousarray(xs.astype(bf16)),
            "w": wT,
            "bvec": bh,
        })
    return in_maps


def run_hw(x, W, b, trace=False, **build_kwargs):
    """Run on 8 NeuronCores; returns (out [N, S] f32, BassKernelResults)."""
    from concourse.bass_utils import run_bass_kernel_spmd
    from concourse.bass_interp import get_hw_module

    nc = _get_nc(**build_kwargs)
    in_maps = make_in_maps(x, W, b)
    old_m = nc.m
    nc.m = get_hw_module(nc.m)
    try:
        res = run_bass_kernel_spmd(
            nc, in_maps, core_ids=list(range(N_CORES)), trace=trace)
    finally:
        nc.m = old_m
    out = np.concatenate(
        [np.asarray(res.results[i]["out"]) for i in range(N_CORES)], axis=0)
    return out.astype(np.float32), res


def kernel(x, W, b):
    out, _ = run_hw(x, W, b, trace=False)
    return out
